# revision 29
# baseline (speedup 1.0000x reference)
"""Trainium2 Bass kernel for nn_MultiHeadAttention (B=2, S=2048, D=2048, H=16).

Sharding: tensor-parallel over heads -- each of the 8 cores owns 2 heads
(both batches) for the q/k/v projections and attention, then two 8-way
AllToAlls (one per local head) convert the head-sharded attention output
Y^T into a token-sharded layout, and each core computes a disjoint
512-token slice of the output projection (no all-reduce needed).

Key structure (evolved against perfetto traces):
- All matmul operands are bf16 (same PE rate as f32r, half the DMA bytes
  and SBUF footprint); psum accumulation stays f32.
- q^T / k^T / v live entirely in SBUF between phases -- projection
  epilogues (DVE bias-add) write straight into persistent tiles.
- Host pre-blocks every DRAM input into the exact [partition][...] layout
  the SBUF tiles want, so each DMA is ~128 large contiguous descriptors.
- Phase order: proj(both heads + V) -> attention(lh0) -> AllToAll(0) ->
  attention(lh1) -> AllToAll(1) -> out-proj.  attention(lh1) runs while
  AllToAll(0) is in flight; dma_start triggers cost ~0.6us each on the
  in-order Sync queue, so DMAs are few and large, and the ya (AllToAll
  output) readback triggers are placed where nothing later on the Sync
  queue is needed sooner.
- Attention is software-pipelined: scores group g+1 issues before AV of
  group g so the PE never waits on the scalar engine's exp.  The softmax
  denominator is accumulated on the DVE in bf16 (2x rate) and contracted
  by ONE ones-matmul per i-tile instead of one per key chunk.
- Out-proj does all lh=0 matmuls first (partials staged in SBUF) so they
  cover the second AllToAll, then lh=1 + combine.
"""

import os
import sys

import numpy as np

_REPO = "/opt/trn_rl_repo"
if _REPO not in sys.path:
    sys.path.insert(0, _REPO)

from concourse import bacc, mybir, tile  # noqa: E402
import concourse.bass as bass  # noqa: E402

B, S, D, H = 2, 2048, 2048, 16
DH = D // H  # 128
NCORES = 8
HPC = H // NCORES  # heads per core = 2
JW = HPC * DH  # per-core head-feature width = 256
T = B * S  # 4096 flattened tokens
TSL = T // NCORES  # per-core output token slice = 512
SCALE = float(np.sqrt(DH))

F32 = mybir.dt.float32
F32R = mybir.dt.float32r
BF16 = mybir.dt.bfloat16
AF = mybir.ActivationFunctionType
ALU = bass.mybir.AluOpType

P = 128
IT = 512  # query i-tile width
NIT = S // IT  # 4 i-tiles per (batch, head)
NJC = S // P  # 16 key chunks per batch
NDC = D // P  # 16 contraction chunks
NTS = T // IT  # 8 token slices (batch 0 first, then batch 1)
XSUB = 8  # x dc-chunks per sub-tile (dma_start triggers cost ~0.6us each,
#          so few big DMAs win; descriptors spread across all 16 queues)
NXS = NDC // XSUB  # 2 x sub-tiles per slice


def build_program():
    nc = bacc.Bacc(
        "TRN2",
        target_bir_lowering=False,
        debug=False,
        num_devices=NCORES,
    )

    # ---- kernel I/O (host pre-blocked; per-core values via in_maps) ----
    xb = nc.dram_tensor("xb", [NTS, P, NDC, IT], BF16, kind="ExternalInput").ap()
    wqb = nc.dram_tensor("wqb", [P, NDC, JW], BF16, kind="ExternalInput").ap()
    wkb = nc.dram_tensor("wkb", [P, NDC, JW], BF16, kind="ExternalInput").ap()
    wvb = nc.dram_tensor("wvb", [P, NDC, JW], BF16, kind="ExternalInput").ap()
    # wo split by key-chunk parity: even chunks feed lh=0, odd feed lh=1
    woE = nc.dram_tensor("woE", [P, NJC // 2, D], BF16, kind="ExternalInput").ap()
    woO = nc.dram_tensor("woO", [P, NJC // 2, D], BF16, kind="ExternalInput").ap()
    bqb = nc.dram_tensor("bqb", [P, HPC], F32, kind="ExternalInput").ap()
    bkb = nc.dram_tensor("bkb", [P, HPC], F32, kind="ExternalInput").ap()
    bvb = nc.dram_tensor("bvb", [P, HPC], F32, kind="ExternalInput").ap()
    bob = nc.dram_tensor("bob", [P, NDC], F32, kind="ExternalInput").ap()
    # 4 diagonal-band mask patterns (1.0 = attend), [p][m][i]
    maskb = nc.dram_tensor("maskb", [P, 4, IT], BF16, kind="ExternalInput").ap()
    onesb = nc.dram_tensor("onesb", [P, P], BF16, kind="ExternalInput").ap()
    out = nc.dram_tensor("out", [P, NDC, TSL], F32, kind="ExternalOutput").ap()

    with tile.TileContext(nc) as tc:
        with (
            tc.tile_pool(name="dram", bufs=1, space="DRAM") as dram,
            tc.tile_pool(name="const", bufs=1) as cpool,
            tc.tile_pool(name="persist", bufs=1) as ppool,
            tc.tile_pool(name="small", bufs=2) as small,
            tc.tile_pool(name="epool", bufs=2) as epool,
        ):
            # ---- persistent SBUF tiles ----
            qT_sb = {}
            kT_sb = {}
            for lh in range(HPC):
                for b in range(B):
                    qT_sb[(lh, b)] = ppool.tile([P, S], BF16, name=f"qT_{lh}_{b}")
                    kT_sb[(lh, b)] = ppool.tile([P, S], BF16, name=f"kT_{lh}_{b}")
            v_sb = {
                b: ppool.tile([P, NJC, JW], BF16, name=f"v_{b}") for b in range(B)
            }
            ya_sb = {
                lh: ppool.tile([P, NCORES, TSL], BF16, name=f"ya{lh}")
                for lh in range(HPC)
            }
            wo_sb = {
                0: ppool.tile([P, NJC // 2, D], BF16, name="woE"),
                1: ppool.tile([P, NJC // 2, D], BF16, name="woO"),
            }

            # per-local-head AllToAll buffers (blocks = dest core's i-slice)
            a2a_in = {
                lh: dram.tile([NCORES, DH, TSL], BF16, name=f"a2a_in_{lh}")
                for lh in range(HPC)
            }
            a2a_out = {
                lh: dram.tile([NCORES * DH, TSL], BF16, name=f"a2a_out_{lh}")
                for lh in range(HPC)
            }

            # ---- constants / weights ----
            # weight DMAs split by dc-pair so they land on parallel queues
            # and the first projection matmul can start within a few us
            wq_w = cpool.tile([P, NDC, JW], BF16)
            wk_w = cpool.tile([P, NDC, JW], BF16)
            wv_w = cpool.tile([P, NDC, JW], BF16)
            for g in range(2):
                sl = slice(g * 8, (g + 1) * 8)
                nc.sync.dma_start(wk_w[:, sl, :], wkb[:, sl, :])
                nc.sync.dma_start(wv_w[:, sl, :], wvb[:, sl, :])
                nc.sync.dma_start(wq_w[:, sl, :], wqb[:, sl, :])
            bq_sb = cpool.tile([P, HPC], F32)
            bk_sb = cpool.tile([P, HPC], F32)
            bv_sb = cpool.tile([P, HPC], F32)
            bo_sb = cpool.tile([P, NDC], F32)
            mask_sb = cpool.tile([P, 4, IT], BF16)
            ones_sb = cpool.tile([P, P], BF16)

            # ---------- projections (SBUF-resident outputs) ----------
            def proj_pass(tag="a", with_v=True):
                """q/k projections for both heads + V, streamed over the
                8 token slices."""
                with (
                    tc.tile_pool(name=f"xpool{tag}", bufs=1) as xpool,
                    tc.tile_pool(name=f"psum_{tag}", bufs=1, space="PSUM") as psp,
                ):
                    for ts in range(NTS):
                        b, lt0 = ts // NIT, (ts % NIT) * IT
                        xs = []
                        for g in range(NXS):
                            xg = xpool.tile(
                                [P, XSUB, IT],
                                BF16,
                                tag="x",
                                bufs=3,
                                name=f"x{tag}_{ts}_{g}",
                            )
                            if with_v and ts == 0 and g == 0:
                                nc.sync.dma_start(
                                    xg[:, 0:2, :], xb[ts, :, 0:2, :]
                                )
                                nc.sync.dma_start(
                                    xg[:, 2:XSUB, :], xb[ts, :, 2:XSUB, :]
                                )
                            else:
                                nc.sync.dma_start(
                                    xg[:], xb[ts, :, g * XSUB : (g + 1) * XSUB, :]
                                )
                            xs.append(xg)
                        if with_v and ts == 0:
                            nc.sync.dma_start(bk_sb[:], bkb)
                            nc.sync.dma_start(bq_sb[:], bqb)
                            nc.sync.dma_start(bv_sb[:], bvb)
                            nc.sync.dma_start(bo_sb[:], bob)
                            nc.sync.dma_start(mask_sb[:], maskb)
                            nc.sync.dma_start(ones_sb[:], onesb)
                        if with_v and ts == 2:
                            nc.sync.dma_start(wo_sb[0][:], woE)
                        if with_v and ts == 4:
                            nc.sync.dma_start(wo_sb[1][:], woO)

                        def xchunk(dc):
                            return xs[dc // XSUB][:, dc % XSUB, :]

                        # accumulation chains advanced together per dc so
                        # 256-row V matmuls hide LDWEIGHTS under 512-row
                        # q/k matmuls
                        # 8 accumulation chains (K h0/h1, Q h0/h1 in
                        # [j, t] psum; V tc0..3 in [t, j] psum), advanced
                        # together per dc so 256-row V matmuls hide their
                        # LDWEIGHTS under 512-row Q/K matmuls
                        pqk = {
                            nm: psp.tile(
                                [P, IT], F32, tag=nm, name=f"p{nm}_{ts}"
                            )
                            for nm in ("k0", "k1", "q0", "q1")
                        }
                        pv = {
                            tc2: psp.tile(
                                [P, JW], F32, tag=f"v{tc2}", name=f"pv{ts}{tc2}"
                            )
                            for tc2 in range(IT // P)
                        }
                        for dc in range(NDC):
                            st, sp = dc == 0, dc == NDC - 1
                            for h in range(HPC):
                                nc.tensor.matmul(
                                    pqk[f"k{h}"][:],
                                    lhsT=wk_w[:, dc, h * DH : (h + 1) * DH],
                                    rhs=xchunk(dc),
                                    start=st,
                                    stop=sp,
                                )
                                nc.tensor.matmul(
                                    pv[h][:],
                                    lhsT=xchunk(dc)[:, h * P : (h + 1) * P],
                                    rhs=wv_w[:, dc, :],
                                    start=st,
                                    stop=sp,
                                )
                                nc.tensor.matmul(
                                    pqk[f"q{h}"][:],
                                    lhsT=wq_w[:, dc, h * DH : (h + 1) * DH],
                                    rhs=xchunk(dc),
                                    start=st,
                                    stop=sp,
                                )
                                nc.tensor.matmul(
                                    pv[2 + h][:],
                                    lhsT=xchunk(dc)[:, (2 + h) * P : (3 + h) * P],
                                    rhs=wv_w[:, dc, :],
                                    start=st,
                                    stop=sp,
                                )
                        # epilogues on DVE: bias add, write bf16 persistents
                        for h in range(HPC):
                            nc.vector.tensor_tensor(
                                kT_sb[(h, b)][:, lt0 : lt0 + IT],
                                pqk[f"k{h}"][:],
                                bk_sb[:, h : h + 1].to_broadcast([P, IT]),
                                ALU.add,
                            )
                            nc.vector.tensor_tensor(
                                qT_sb[(h, b)][:, lt0 : lt0 + IT],
                                pqk[f"q{h}"][:],
                                bq_sb[:, h : h + 1].to_broadcast([P, IT]),
                                ALU.add,
                            )
                        for tc2 in range(IT // P):
                            # v bias deferred to the attention epilogue
                            # (softmax rows sum to 1: attn@(v+b) = attn@v + b)
                            nc.vector.tensor_copy(
                                v_sb[b][:, lt0 // P + tc2, :], pv[tc2][:]
                            )

            # ---------- attention for one local head + its AllToAll ----------
            def attention(lh, post_b0=None):
                with (
                    tc.tile_pool(name=f"psS{lh}", bufs=2, space="PSUM") as psS,
                    tc.tile_pool(name=f"psO{lh}", bufs=2, space="PSUM") as psO,
                    tc.tile_pool(name=f"psR{lh}", bufs=2, space="PSUM") as psR,
                ):
                    # softmax denominators: exp chunks accumulate into racc
                    # on the DVE (bf16, 2x rate), then ONE ones-matmul per
                    # i-tile contracts racc's 128 partitions.  That matmul +
                    # epilogue are emitted after the next tile's first
                    # scores group so the PE never waits on the DVE tail.
                    pending = None

                    def flush_pending():
                        nonlocal pending
                        if pending is None:
                            return
                        racc, po, pr, b, it = pending
                        pending = None
                        if racc is not None:
                            nc.tensor.matmul(
                                pr[:],
                                lhsT=ones_sb[:],
                                rhs=racc[:],
                                start=False,
                                stop=True,
                            )
                        rinv = small.tile(
                            [P, IT], F32, tag="rinv", name=f"ri{lh}{b}{it}"
                        )
                        nc.vector.reciprocal_approx_fast(rinv[:], pr[:])
                        y_sb = small.tile([P, IT], BF16, tag="y", name=f"y{lh}{b}{it}")
                        nc.vector.tensor_tensor(y_sb[:], po[:], rinv[:], ALU.mult)
                        nc.vector.tensor_tensor(
                            y_sb[:],
                            y_sb[:],
                            bv_sb[:, lh : lh + 1].to_broadcast([P, IT]),
                            ALU.add,
                        )
                        g = NIT * b + it  # destination core / a2a block
                        nc.sync.dma_start(a2a_in[lh][g, :, :], y_sb[:])

                    for b in range(B):
                        if b == 1 and post_b0 is not None:
                            post_b0()
                        kT = kT_sb[(lh, b)]
                        for it in range(NIT):
                            q_ap = qT_sb[(lh, b)][:, it * IT : (it + 1) * IT]
                            njc = (it + 1) * (IT // P)
                            po = psO.tile([P, IT], F32, tag="o", name=f"po{lh}{b}{it}")
                            pr = psR.tile([P, IT], F32, tag="r", name=f"pr{lh}{b}{it}")
                            # the i-tile that gates this head's AllToAll gets
                            # its whole denominator from direct PE matmuls --
                            # no DVE dependency in the trigger tail
                            d_all = b == B - 1 and it == NIT - 1
                            racc = (
                                None
                                if d_all
                                else small.tile(
                                    [P, IT], BF16, tag="racc", name=f"ra{lh}{b}{it}"
                                )
                            )

                            def emit_av(e_tile, jg):
                                for k2 in range(2):
                                    jc = jg * 2 + k2
                                    nc.tensor.matmul(
                                        po[:],
                                        lhsT=v_sb[b][:, jc, lh * DH : (lh + 1) * DH],
                                        rhs=e_tile[:, k2, :],
                                        start=(jc == 0),
                                        stop=(jc == njc - 1),
                                    )
                                    if d_all:
                                        nc.tensor.matmul(
                                            pr[:],
                                            lhsT=ones_sb[:],
                                            rhs=e_tile[:, k2, :],
                                            start=(jc == 0),
                                            stop=(jc == njc - 1),
                                        )
                                    elif jc % 4 == 0:
                                        nc.tensor.matmul(
                                            pr[:],
                                            lhsT=ones_sb[:],
                                            rhs=e_tile[:, k2, :],
                                            start=(jc == 0),
                                            stop=False,
                                        )

                            prev = None
                            for jg in range(njc // 2):
                                ps2 = psS.tile([P, 2, IT], F32, tag="s")
                                for k2 in range(2):
                                    jc = jg * 2 + k2
                                    nc.tensor.matmul(
                                        ps2[:, k2, :],
                                        lhsT=kT[:, jc * P : (jc + 1) * P],
                                        rhs=q_ap,
                                        start=True,
                                        stop=True,
                                    )
                                if jg == 0:
                                    # prev i-tile's rowsum matmul slots in
                                    # behind this tile's first scores
                                    flush_pending()
                                e_sb = epool.tile([P, 2, IT], BF16, tag="e", bufs=3)
                                nc.scalar.activation(
                                    e_sb[:], ps2[:], AF.Exp, scale=1.0 / SCALE
                                )
                                for k2 in range(2):
                                    jc = jg * 2 + k2
                                    if jc >= (it * IT) // P:
                                        m = jc - (it * IT) // P
                                        nc.vector.tensor_tensor(
                                            e_sb[:, k2, :],
                                            e_sb[:, k2, :],
                                            mask_sb[:, m, :],
                                            ALU.mult,
                                        )
                                    if racc is None:
                                        pass
                                    elif jc == 1:
                                        nc.vector.tensor_copy(
                                            racc[:], e_sb[:, k2, :]
                                        )
                                    elif jc % 4 != 0:
                                        nc.vector.tensor_tensor(
                                            racc[:], racc[:], e_sb[:, k2, :], ALU.add
                                        )
                                if prev is not None:
                                    emit_av(*prev)
                                prev = (e_sb, jg)
                            emit_av(*prev)
                            pending = (racc, po, pr, b, it)
                            if d_all:
                                flush_pending()  # inline: tail has no DVE dep
                    flush_pending()  # before this lh's collective
                nc.gpsimd.collective_compute(
                    "AllToAll",
                    ALU.bypass,
                    replica_groups=[list(range(NCORES))],
                    ins=[a2a_in[lh][:].opt()],
                    outs=[a2a_out[lh][:].opt()],
                )


            def ya_dma(lh):
                # emitted where the in-order Sync queue has slack: this
                # trigger waits for collective(lh), so nothing later on the
                # queue may be needed sooner
                nc.sync.dma_start(
                    ya_sb[lh][:],
                    a2a_out[lh][:].rearrange("(s p) i -> p s i", p=P),
                )

            proj_pass()
            attention(0)
            attention(1, post_b0=lambda: ya_dma(0))
            ya_dma(1)

            # ---------- output projection on own token slice ----------
            # ya_sb[lh] block s holds key chunk jc = 2s + lh, i.e. the s-th
            # chunk of wo_sb[lh] (parity-split).  ALL lh=0 matmuls run first
            # (partials staged to SBUF) so they cover the lh=1 AllToAll;
            # lh=1 matmuls then reuse the psum banks and the DVE combines
            # partial + psum + bias.
            with (
                tc.tile_pool(name="opart", bufs=1) as opart,
                tc.tile_pool(name="ostage", bufs=2) as ostage,
                tc.tile_pool(name="psout", bufs=4, space="PSUM") as psout,
            ):
                EG = 2  # e-chunks per psum tile
                NEG = NDC // EG
                parts = [
                    opart.tile([P, EG, TSL], BF16, name=f"part{eg}")
                    for eg in range(NEG)
                ]

                def emit_mms(lh, eg, ps):
                    for sub in range(EG):
                        ec = eg * EG + sub
                        for s in range(NCORES):
                            nc.tensor.matmul(
                                ps[:, sub, :],
                                lhsT=wo_sb[lh][:, s, ec * P : ec * P + P],
                                rhs=ya_sb[lh][:, s, :],
                                start=(s == 0),
                                stop=(s == NCORES - 1),
                            )

                # pass 1: lh=0 into psum, drain raw partials to SBUF
                for eg in range(NEG):
                    ps = psout.tile([P, EG, TSL], F32, tag="out", name=f"p0_{eg}")
                    emit_mms(0, eg, ps)
                    nc.vector.tensor_copy(parts[eg][:], ps[:])
                # pass 2: lh=1 into psum, combine with partial + bias, store
                for eg in range(NEG):
                    ps = psout.tile([P, EG, TSL], F32, tag="out", name=f"p1_{eg}")
                    emit_mms(1, eg, ps)
                    ost = ostage.tile([P, EG, TSL], F32, tag="ost", name=f"os{eg}")
                    nc.vector.tensor_tensor(ost[:], ps[:], parts[eg][:], ALU.add)
                    nc.vector.tensor_tensor(
                        ost[:],
                        ost[:],
                        bo_sb[:, eg * EG : (eg + 1) * EG, None].to_broadcast(
                            [P, EG, TSL]
                        ),
                        ALU.add,
                    )
                    nc.sync.dma_start(out[:, eg * EG : (eg + 1) * EG, :], ost[:])

    nc.finalize()  # bacc compile: regalloc etc. -- required before execution
    return nc


_PROGRAM = None


def _get_program():
    global _PROGRAM
    if _PROGRAM is None:
        _PROGRAM = build_program()
    return _PROGRAM


def _host_prep(x, mask, wq, bq, wk, bk, wv, bv, wo, bo):
    """Build the 8 per-core input maps (host-side marshalling only)."""
    import ml_dtypes

    f = np.float32
    bf = ml_dtypes.bfloat16
    x2 = np.asarray(x, dtype=f).reshape(T, D)
    # [ts][p][dc][t] blocked x^T so every DMA descriptor is contiguous
    xb = x2.T.reshape(NDC, P, NTS, IT).transpose(2, 1, 0, 3).astype(bf)

    woT = np.asarray(wo, dtype=f).T.reshape(NJC, P, D)  # [jc][p][e]
    woE = woT[0::2].transpose(1, 0, 2).astype(bf)  # [p][s][e], jc = 2s
    woO = woT[1::2].transpose(1, 0, 2).astype(bf)  # [p][s][e], jc = 2s+1
    bo_b = np.ascontiguousarray(np.asarray(bo, dtype=f).reshape(NDC, P).T)

    # diagonal-band mask patterns from the provided mask (True = masked out)
    mask_np = np.asarray(mask)
    maskp = np.empty((4, P, IT), dtype=f)
    for m in range(4):
        maskp[m] = (~mask_np[0:IT, m * P : (m + 1) * P]).T.astype(f)
    maskb = maskp.transpose(1, 0, 2).astype(bf)  # [p][m][i]

    wq_, wk_, wv_ = (np.asarray(w, dtype=f) for w in (wq, wk, wv))
    bq_, bk_, bv_ = (np.asarray(v_, dtype=f) for v_ in (bq, bk, bv))

    in_maps = []
    for c in range(NCORES):
        j0, j1 = c * JW, (c + 1) * JW
        in_maps.append(
            {
                "xb": xb,
                "wqb": wq_[j0:j1, :].T.reshape(NDC, P, JW).transpose(1, 0, 2).astype(bf),
                "wkb": wk_[j0:j1, :].T.reshape(NDC, P, JW).transpose(1, 0, 2).astype(bf),
                "wvb": wv_[j0:j1, :].T.reshape(NDC, P, JW).transpose(1, 0, 2).astype(bf),
                "woE": woE,
                "woO": woO,
                "bqb": np.ascontiguousarray(bq_[j0:j1].reshape(HPC, P).T),
                "bkb": np.ascontiguousarray(bk_[j0:j1].reshape(HPC, P).T),
                "bvb": np.ascontiguousarray(bv_[j0:j1].reshape(HPC, P).T),
                "bob": bo_b,
                "maskb": maskb,
                "onesb": np.ones((P, P), dtype=bf),
            }
        )
    return in_maps


LAST_RESULTS = None  # BassKernelResults of the most recent run (for test.py)


def _assemble(per_core_outs):
    """[P, NDC, TSL] blocked slices -> full [B, S, D] output."""
    outT = np.concatenate(
        [
            np.asarray(o, dtype=np.float32).transpose(1, 0, 2).reshape(D, TSL)
            for o in per_core_outs
        ],
        axis=1,
    )
    return np.ascontiguousarray(outT.T).reshape(B, S, D).astype(np.float32)


def kernel(x, mask, wq, bq, wk, bk, wv, bv, wo, bo):
    global LAST_RESULTS
    from concourse.bass_utils import run_bass_kernel_spmd

    nc = _get_program()
    in_maps = _host_prep(x, mask, wq, bq, wk, bk, wv, bv, wo, bo)
    trace = os.environ.get("KERNEL_TRACE", "") == "1"
    kwargs = {}
    if os.environ.get("KERNEL_TRACE_ALL", "") == "1":
        kwargs["trace_cores"] = list(range(NCORES))
        kwargs["stitch_traces"] = True
    res = run_bass_kernel_spmd(
        nc, in_maps, core_ids=list(range(NCORES)), trace=trace, **kwargs
    )
    LAST_RESULTS = res
    return _assemble([res.results[c]["out"] for c in range(NCORES)])
